# revision 25
# baseline (speedup 1.0000x reference)
# Bass/Tile Trainium2 kernel for nn_Attention_48816598286380.
#
# Reference computation (B=4, N=512, M=8192, Hq=512, Ck=256, H=8, D=64):
#   q = x @ Wq;  k,v = split(context @ Wkv);  per-head softmax(q k^T / sqrt(D)) v
#   out = attn_out @ Wo + bo
#
# Sharding: 8 cores = 4 batches x 2 head-groups (4 heads each).  Each core
# computes its batch's attention for its 4 heads plus the partial output
# projection over those heads; the host sums the two partial projections per
# batch (pure unshard of a sum-sharded tensor; bo is split half/half so the
# sum carries the full bias).
#
# On-device layout: everything is computed in "transposed" orientation so all
# matmul contractions sit on the partition axis:
#   qT[d, n], kT[d, m] from xT / contextT (host supplies the transposes)
#   scoresT[m, n] = kT(m-tile).T @ qT      (two heads packed via PE row tiling)
#   E = exp(scoresT / 8)  on ScalarE, PSUM -> SBUF, [128, 1024] per instr
#   numerT[d, n] (+ denominator row 64) = v_aug(m-tile).T @ E  accumulated in
#     PSUM, where v_aug = [v | ones], so the softmax denominator falls out of
#     the same matmul.
#   out_partial[n, f] = sum_h (numerT_h / den_h) contracted with Wo rows.
#
# All matmul-feeding tensors are declared float32r (full-rate fp32 path on
# the PE; plain fp32 runs at 1/4 rate; walrus requires producers to declare
# f32r output).  Two passes over m (one per head pair) keep the PSUM
# budget at 8 banks; kT/v production is software-pipelined one chunk ahead of
# the attention loop, and pair 1's kT plus all of v stay resident in SBUF so
# pass 1 needs no DMA or production work.

import numpy as np

B, N, M = 4, 512, 8192
QUERY_DIM, INPUT_DIM = 512, 256
HEADS, DIM_HEAD = 8, 64
ATT_DIM = HEADS * DIM_HEAD  # 512
HPC = 4          # heads per core
N_CORES = 8
MCHUNK = 1024    # context rows produced per pipeline step
NCHUNKS = M // MCHUNK
MT_PER_CHUNK = MCHUNK // 128
SCALE = DIM_HEAD ** -0.5

_CACHE = {}


def _build_nc():
    import concourse.bacc as bacc
    import concourse.bass as bass
    import concourse.mybir as mybir
    import concourse.tile as tile

    f32 = mybir.dt.float32
    f32r = mybir.dt.float32r
    EXP = mybir.ActivationFunctionType.Exp

    nc = bacc.Bacc(None, target_bir_lowering=False)

    ct = nc.dram_tensor("ct", [INPUT_DIM, M], f32r, kind="ExternalInput")  # context[b].T
    xt = nc.dram_tensor("xt", [QUERY_DIM, N], f32r, kind="ExternalInput")  # x[b].T
    wq = nc.dram_tensor("wq", [QUERY_DIM, HPC * DIM_HEAD], f32r, kind="ExternalInput")
    wk = nc.dram_tensor("wk", [INPUT_DIM, HPC * DIM_HEAD], f32r, kind="ExternalInput")
    wv = nc.dram_tensor("wv", [INPUT_DIM, HPC * DIM_HEAD], f32r, kind="ExternalInput")
    wo = nc.dram_tensor("wo", [DIM_HEAD, HPC, QUERY_DIM], f32r, kind="ExternalInput")
    bo2 = nc.dram_tensor("bo2", [1, QUERY_DIM], f32, kind="ExternalInput")  # bo / 2
    out = nc.dram_tensor("out", [N, QUERY_DIM], f32, kind="ExternalOutput")

    ct_r = ct[:, :].rearrange("(t p) m -> p t m", p=128)    # [128, 2, M]
    xt_r = xt[:, :].rearrange("(t p) n -> p t n", p=128)    # [128, 4, N]
    wq_r = wq[:, :].rearrange("(t p) d -> p t d", p=128)    # [128, 4, 256]
    wk_r = wk[:, :].rearrange("(t p) d -> p t d", p=128)    # [128, 2, 256]
    wv_r = wv[:, :].rearrange("(t p) d -> p t d", p=128)    # [128, 2, 256]
    out_r = out[:, :].rearrange("(t p) f -> p t f", p=128)  # [128, 4, 512]

    with tile.TileContext(nc) as tc:
        with (
            tc.tile_pool(name="const", bufs=1) as cp,
            tc.tile_pool(name="ctp", bufs=2) as ctp,
            tc.tile_pool(name="ktp", bufs=2) as ktp,
            tc.tile_pool(name="ep", bufs=4) as ep,
            tc.tile_pool(name="scp", bufs=3, space="PSUM") as scp,
            tc.tile_pool(name="accp", bufs=1, space="PSUM") as accp,
        ):
            # ---- constants ----
            xt_sb = cp.tile([128, 4, N], f32r)
            wq_sb = cp.tile([128, 4, HPC * DIM_HEAD], f32r)
            wk_sb = cp.tile([128, 2, HPC * DIM_HEAD], f32r)
            wv_sb = cp.tile([128, 2, HPC * DIM_HEAD], f32r)
            wo_sb = cp.tile([DIM_HEAD, HPC, QUERY_DIM], f32r)
            bo_sb = cp.tile([1, QUERY_DIM], f32)
            bo_bc = cp.tile([128, QUERY_DIM], f32)
            qt_sb = cp.tile([128, 2, N], f32r)
            # v for all 4 heads, all of M, with a ones column per head:
            # [128 (m within tile), m-tile, head, 64 v | 1 one]
            v_full = cp.tile([128, M // 128, HPC, DIM_HEAD + 1], f32r)
            stack_sb = cp.tile([DIM_HEAD, HPC, N], f32r)  # normalized attn outT
            recip_sb = cp.tile([128, 2, N], f32)          # partition 64, per pass
            recip0_sb = cp.tile([1, 2, N], f32)           # moved to partition 0
            bcast_sb = cp.tile([DIM_HEAD, 2, N], f32)
            out0_sb = cp.tile([128, 4, QUERY_DIM], f32)   # pair-0 proj + bias
            out_sb = cp.tile([128, 4, QUERY_DIM], f32)

            # prologue DMAs, ordered so the first production chunk and qT can
            # start as early as possible; the rest of the prologue (wv, ones,
            # wo, bias) is emitted after chunk 0's context DMA below.
            nc.sync.dma_start(out=wq_sb[:], in_=wq_r)
            nc.sync.dma_start(out=xt_sb[:], in_=xt_r)
            nc.sync.dma_start(out=wk_sb[:], in_=wk_r)

            # kT for pair 1 stays resident; pair 0's kT lives in rotating
            # chunk tiles consumed during pass 0.
            kt_f1 = ktp.tile([128, M], f32r, tag="ktf1", name="ktf1", bufs=1)
            kt_chunks = {}

            def produce_chunk(mc):
                """DMA chunk mc of contextT; return kT/v production emitters
                (closures) so production interleaves with attention tiles."""
                ct_t = ctp.tile([128, 2, MCHUNK], f32r, tag="ct", name=f"ct{mc}")
                ct_dma = nc.sync.dma_start(
                    out=ct_t[:], in_=ct_r[:, :, mc * MCHUNK:(mc + 1) * MCHUNK]
                )
                if mc >= 1:
                    # keep the small prologue DMAs ahead of the chunk stream
                    # on the SP queue
                    for d in late_dmas:
                        tile.add_dep_helper(ct_dma.ins, d.ins, sync=False,
                                            reason="prologue before ct stream")
                kt_t = ktp.tile([128, MCHUNK], f32r, tag="kt", name=f"kt{mc}")
                kt_chunks[mc] = kt_t

                def kt_group(pp):
                    def go():
                        kt_ps = scp.tile([128, 1024], f32, tag="sc",
                                         name=f"ktps{pp}{mc}")
                        for h2 in range(2):
                            for t in range(2):
                                nc.tensor.matmul(
                                    kt_ps[:, h2 * 512:(h2 + 1) * 512],
                                    lhsT=wk_sb[:, t, pp * 128:(pp + 1) * 128],
                                    rhs=ct_t[:, t, h2 * 512:(h2 + 1) * 512],
                                    start=(t == 0), stop=(t == 1),
                                    skip_group_check=True,
                                )
                        dst = (kt_t[:, :] if pp == 0 else
                               kt_f1[:, mc * MCHUNK:(mc + 1) * MCHUNK])
                        nc.vector.tensor_copy(dst, kt_ps[:])
                    return go

                def v_group(s4):
                    def go():
                        v_ps = scp.tile([128, 1024], f32, tag="sc",
                                        name=f"vps{mc}{s4}")
                        for q in range(4):
                            s = s4 * 4 + q
                            for t in range(2):
                                nc.tensor.matmul(
                                    v_ps[:, q * 256:(q + 1) * 256],
                                    lhsT=ct_t[:, t, s * 128:(s + 1) * 128],
                                    rhs=wv_sb[:, t, :],
                                    start=(t == 0), stop=(t == 1),
                                    skip_group_check=True,
                                )
                        nc.vector.tensor_copy(
                            v_full[:, mc * MT_PER_CHUNK + s4 * 4:
                                   mc * MT_PER_CHUNK + s4 * 4 + 4, :, 0:DIM_HEAD],
                            v_ps[:].rearrange("p (s h d) -> p s h d", s=4, h=HPC),
                        )
                    return go

                # order: pair-0 kT first (needed immediately), v next (needed
                # by AV shortly after), pair-1 kT last (pass 1 only)
                return [kt_group(0), v_group(0), v_group(1), kt_group(1)]

            def attention_tile(p, mi, acc):
                sc = scp.tile([128, 1024], f32, tag="sc", name=f"sc{p}{mi}")
                ks = (kt_chunks[mi // MT_PER_CHUNK] if p == 0 else kt_f1)
                off = (mi % MT_PER_CHUNK if p == 0 else mi) * 128
                ks = ks[:, off:off + 128]
                # two heads in one PE pass via row tiling
                nc.tensor.matmul(sc[:, 0:512], lhsT=ks[0:64, :],
                                 rhs=qt_sb[0:64, p, :], start=True, stop=True)
                nc.tensor.matmul(sc[:, 512:1024], lhsT=ks[64:128, :],
                                 rhs=qt_sb[64:128, p, :], start=True, stop=True)
                e_t = ep.tile([128, 1024], f32r, tag="e", name=f"e{p}{mi}")
                nc.scalar.activation(e_t[:], sc[:], EXP, scale=SCALE)
                for h2 in range(2):
                    nc.tensor.matmul(
                        acc[h2][0:DIM_HEAD + 1, :],
                        lhsT=v_full[:, mi, 2 * p + h2, :],
                        rhs=e_t[:, h2 * 512:(h2 + 1) * 512],
                        start=(mi == 0), stop=(mi == M // 128 - 1),
                        skip_group_check=True,
                    )

            def pass_tail(p, acc):
                """normalize numerators by the ones-row denominator; per-head
                chains so head 0's broadcast overlaps head 1's reciprocal"""
                for h2 in range(2):
                    nc.vector.reciprocal(
                        recip_sb[DIM_HEAD:DIM_HEAD + 1, h2, :],
                        acc[h2][DIM_HEAD:DIM_HEAD + 1, :],
                    )
                    # partition_broadcast on HW always reads partition 0 of
                    # the tensor, so shift the reciprocal row there first.
                    nc.sync.dma_start(
                        out=recip0_sb[0:1, h2, :],
                        in_=recip_sb[DIM_HEAD:DIM_HEAD + 1, h2, :],
                    )
                    nc.gpsimd.partition_broadcast(bcast_sb[:, h2, :],
                                                  recip0_sb[0:1, h2, :])
                    nc.vector.tensor_mul(
                        stack_sb[:, 2 * p + h2, :], acc[h2][0:DIM_HEAD, :],
                        bcast_sb[:, h2, :]
                    )

            # chunk-0 context DMA goes out right behind the qT weights
            chunk0 = produce_chunk(0)

            # late prologue (not needed until mid-kernel)
            late_dmas = []
            late_dmas.append(nc.sync.dma_start(out=wv_sb[:], in_=wv_r))
            late_dmas.append(nc.sync.dma_start(out=wo_sb[:], in_=wo[:, :, :]))
            late_dmas.append(nc.sync.dma_start(out=bo_sb[:], in_=bo2[:, :]))
            # ones column of v_aug: memset a [128, 1] column, then one
            # broadcast-copy into the strided ones slots (rounds to f32r)
            ones_col = cp.tile([128, 1], f32)
            nc.vector.memset(ones_col[:], 1.0)
            _oc, _vdst = bass.broadcast_tensor_aps(
                ones_col[:, :], v_full[:, :, :, DIM_HEAD].rearrange(
                    "p s h -> p (s h)")[:, None, :].rearrange("p o q -> p (o q)")
            )
            nc.vector.tensor_copy(_vdst, _oc)
            nc.gpsimd.partition_broadcast(bo_bc[:], bo_sb[0:1, :])

            # qT per head-pair p: [128, N]; rows 0-63 head 2p, 64-127 head 2p+1
            q_ps = scp.tile([128, 1024], f32, tag="sc", name="q_ps")
            for p in range(2):
                for t in range(4):
                    nc.tensor.matmul(
                        q_ps[:, p * 512:(p + 1) * 512],
                        lhsT=wq_sb[:, t, p * 128:(p + 1) * 128],
                        rhs=xt_sb[:, t, :],
                        start=(t == 0), stop=(t == 3),
                        skip_group_check=True,
                    )
            nc.vector.tensor_copy(
                qt_sb[:, :, :], q_ps[:].rearrange("p (a n) -> p a n", a=2))

            # ---- pass 0 (heads 0,1), production pipelined one chunk ahead --
            acc0 = [accp.tile([128, N], f32, tag=f"acc{h2}", name=f"a0{h2}")
                    for h2 in range(2)]
            for step in range(NCHUNKS + 1):
                prod = (chunk0 if step == 0 else produce_chunk(step)) \
                    if step < NCHUNKS else []
                atts = (
                    list(range((step - 1) * MT_PER_CHUNK, step * MT_PER_CHUNK))
                    if step >= 1 else []
                )
                for i in range(max(2 * len(prod), len(atts))):
                    if i < len(atts):
                        attention_tile(0, atts[i], acc0)
                    if i % 2 == 1 and i // 2 < len(prod):
                        prod[i // 2]()
            pass_tail(0, acc0)

            # partial projection for pair 0 (+ bias) overlaps pass 1
            def proj_pair0():
                for g in range(2):
                    pr0 = scp.tile([128, 1024], f32, tag="sc", name=f"pr0{g}")
                    for j in range(2):
                        nt = g * 2 + j
                        for h in range(2):
                            nc.tensor.matmul(
                                pr0[:, j * 512:(j + 1) * 512],
                                lhsT=stack_sb[:, h, nt * 128:(nt + 1) * 128],
                                rhs=wo_sb[:, h, :],
                                start=(h == 0), stop=(h == 1),
                                skip_group_check=True,
                            )
                    for j in range(2):
                        nt = g * 2 + j
                        nc.vector.tensor_add(
                            out0_sb[:, nt, :], pr0[:, j * 512:(j + 1) * 512],
                            bo_bc[:])

            # ---- pass 1 (heads 2,3): pure attention from resident kT/v ----
            acc1 = [accp.tile([128, N], f32, tag=f"acc{h2}", name=f"a1{h2}")
                    for h2 in range(2)]
            for mi in range(M // 128):
                attention_tile(1, mi, acc1)
                if mi == 8:
                    proj_pair0()
            pass_tail(1, acc1)

            # ---- pair-1 projection + combine + store ----
            for g in range(2):
                pr = scp.tile([128, 1024], f32, tag="sc", name=f"pr{g}")
                for j in range(2):
                    nt = g * 2 + j
                    for h in range(2, 4):
                        nc.tensor.matmul(
                            pr[:, j * 512:(j + 1) * 512],
                            lhsT=stack_sb[:, h, nt * 128:(nt + 1) * 128],
                            rhs=wo_sb[:, h, :],
                            start=(h == 2), stop=(h == 3),
                            skip_group_check=True,
                        )
                for j in range(2):
                    nt = g * 2 + j
                    nc.vector.tensor_add(
                        out_sb[:, nt, :], pr[:, j * 512:(j + 1) * 512],
                        out0_sb[:, nt, :])
                    nc.sync.dma_start(out=out_r[:, nt, :], in_=out_sb[:, nt, :])

    nc.compile()
    return nc


def _get_nc():
    if "nc" not in _CACHE:
        _CACHE["nc"] = _build_nc()
    return _CACHE["nc"]


def _make_in_maps(x, context, Wq, Wkv, Wo, bo):
    x = np.asarray(x, dtype=np.float32)
    context = np.asarray(context, dtype=np.float32)
    Wq = np.asarray(Wq, dtype=np.float32)
    Wkv = np.asarray(Wkv, dtype=np.float32)
    Wo = np.asarray(Wo, dtype=np.float32)
    bo = np.asarray(bo, dtype=np.float32)

    Wk = Wkv[:, :ATT_DIM]
    Wv = Wkv[:, ATT_DIM:]
    bo2 = np.ascontiguousarray((bo / 2.0)[None, :])

    in_maps = []
    for c in range(N_CORES):
        b, g = divmod(c, 2)
        hs = g * HPC * DIM_HEAD           # column offset of this core's heads
        he = hs + HPC * DIM_HEAD
        wo_core = Wo[hs:he, :].reshape(HPC, DIM_HEAD, QUERY_DIM)
        in_maps.append({
            "ct": np.ascontiguousarray(context[b].T),
            "xt": np.ascontiguousarray(x[b].T),
            "wq": np.ascontiguousarray(Wq[:, hs:he]),
            "wk": np.ascontiguousarray(Wk[:, hs:he]),
            "wv": np.ascontiguousarray(Wv[:, hs:he]),
            "wo": np.ascontiguousarray(wo_core.transpose(1, 0, 2)),
            "bo2": bo2,
        })
    return in_maps


def run(inputs, trace=False, **spmd_kwargs):
    """Run the kernel; returns (full_output [B,N,QUERY_DIM], BassKernelResults)."""
    from concourse.bass_utils import run_bass_kernel_spmd

    nc = _get_nc()
    in_maps = _make_in_maps(**inputs)
    res = run_bass_kernel_spmd(
        nc, in_maps, core_ids=list(range(N_CORES)), trace=trace, **spmd_kwargs
    )
    outs = [r["out"] for r in res.results]
    full = np.empty((B, N, QUERY_DIM), dtype=np.float32)
    for b in range(B):
        full[b] = outs[2 * b] + outs[2 * b + 1]
    return full, res


def kernel(**inputs) -> np.ndarray:
    full, _ = run(inputs, trace=False)
    return full


# revision 34
# speedup vs baseline: 1.0009x; 1.0009x over previous
# Bass/Tile Trainium2 kernel for nn_Attention_48816598286380.
#
# Reference computation (B=4, N=512, M=8192, Hq=512, Ck=256, H=8, D=64):
#   q = x @ Wq;  k,v = split(context @ Wkv);  per-head softmax(q k^T / sqrt(D)) v
#   out = attn_out @ Wo + bo
#
# Sharding: 8 cores = 4 batches x 2 head-groups (4 heads each).  Each core
# computes its batch's attention for its 4 heads plus the partial output
# projection over those heads; the host sums the two partial projections per
# batch (pure unshard of a sum-sharded tensor; bo is split half/half so the
# sum carries the full bias).
#
# On-device layout: everything is computed in "transposed" orientation so all
# matmul contractions sit on the partition axis:
#   qT[d, n], kT[d, m] from xT / contextT (host supplies the transposes)
#   scoresT[m, n] = kT(m-tile).T @ qT      (two heads packed via PE row tiling)
#   E = exp(scoresT / 8)  on ScalarE, PSUM -> SBUF, [128, 1024] per instr
#   numerT[d, n] (+ denominator row 64) = v_aug(m-tile).T @ E  accumulated in
#     PSUM, where v_aug = [v | ones], so the softmax denominator falls out of
#     the same matmul.
#   out_partial[n, f] = sum_h (numerT_h / den_h) contracted with Wo rows.
#
# All matmul-feeding tensors are declared float32r (full-rate fp32 path on
# the PE; plain fp32 runs at 1/4 rate; walrus requires producers to declare
# f32r output).  Two passes over m (one per head pair) keep the PSUM
# budget at 8 banks; kT/v production is software-pipelined one chunk ahead of
# the attention loop, and pair 1's kT plus all of v stay resident in SBUF so
# pass 1 needs no DMA or production work.

import numpy as np

B, N, M = 4, 512, 8192
QUERY_DIM, INPUT_DIM = 512, 256
HEADS, DIM_HEAD = 8, 64
ATT_DIM = HEADS * DIM_HEAD  # 512
HPC = 4          # heads per core
N_CORES = 8
MCHUNK = 1024    # context rows produced per pipeline step
NCHUNKS = M // MCHUNK
MT_PER_CHUNK = MCHUNK // 128
SCALE = DIM_HEAD ** -0.5

_CACHE = {}


def _build_nc():
    import concourse.bacc as bacc
    import concourse.bass as bass
    import concourse.mybir as mybir
    import concourse.tile as tile

    f32 = mybir.dt.float32
    f32r = mybir.dt.float32r
    EXP = mybir.ActivationFunctionType.Exp

    nc = bacc.Bacc(None, target_bir_lowering=False)

    ct = nc.dram_tensor("ct", [INPUT_DIM, M], f32r, kind="ExternalInput")  # context[b].T
    xt = nc.dram_tensor("xt", [QUERY_DIM, N], f32r, kind="ExternalInput")  # x[b].T
    wq = nc.dram_tensor("wq", [QUERY_DIM, HPC * DIM_HEAD], f32r, kind="ExternalInput")
    wk = nc.dram_tensor("wk", [INPUT_DIM, HPC * DIM_HEAD], f32r, kind="ExternalInput")
    wv = nc.dram_tensor("wv", [INPUT_DIM, HPC * DIM_HEAD], f32r, kind="ExternalInput")
    wo = nc.dram_tensor("wo", [DIM_HEAD, HPC, QUERY_DIM], f32r, kind="ExternalInput")
    bo2 = nc.dram_tensor("bo2", [1, QUERY_DIM], f32, kind="ExternalInput")  # bo / 2
    out = nc.dram_tensor("out", [N, QUERY_DIM], f32, kind="ExternalOutput")

    ct_r = ct[:, :].rearrange("(t p) m -> p t m", p=128)    # [128, 2, M]
    xt_r = xt[:, :].rearrange("(t p) n -> p t n", p=128)    # [128, 4, N]
    wq_r = wq[:, :].rearrange("(t p) d -> p t d", p=128)    # [128, 4, 256]
    wk_r = wk[:, :].rearrange("(t p) d -> p t d", p=128)    # [128, 2, 256]
    wv_r = wv[:, :].rearrange("(t p) d -> p t d", p=128)    # [128, 2, 256]
    out_r = out[:, :].rearrange("(t p) f -> p t f", p=128)  # [128, 4, 512]

    with tile.TileContext(nc) as tc:
        with (
            tc.tile_pool(name="const", bufs=1) as cp,
            tc.tile_pool(name="ctp", bufs=2) as ctp,
            tc.tile_pool(name="ktp", bufs=2) as ktp,
            tc.tile_pool(name="ep", bufs=4) as ep,
            tc.tile_pool(name="scp", bufs=3, space="PSUM") as scp,
            tc.tile_pool(name="accp", bufs=1, space="PSUM") as accp,
        ):
            # ---- constants ----
            xt_sb = cp.tile([128, 4, N], f32r)
            wq_sb = cp.tile([128, 4, HPC * DIM_HEAD], f32r)
            wk_sb = cp.tile([128, 2, HPC * DIM_HEAD], f32r)
            wv_sb = cp.tile([128, 2, HPC * DIM_HEAD], f32r)
            wo_sb = cp.tile([DIM_HEAD, HPC, QUERY_DIM], f32r)
            bo_sb = cp.tile([1, QUERY_DIM], f32)
            bo_bc = cp.tile([128, QUERY_DIM], f32)
            qt_sb = cp.tile([128, 2, N], f32r)
            # v for all 4 heads, all of M, with a ones column per head:
            # [128 (m within tile), m-tile, head, 64 v | 1 one]
            v_full = cp.tile([128, M // 128, HPC, DIM_HEAD + 1], f32r)
            stack_sb = cp.tile([DIM_HEAD, HPC, N], f32r)  # normalized attn outT
            recip_sb = cp.tile([128, 2, N], f32)          # partition 64, per pass
            recip0_sb = cp.tile([1, 2, N], f32)           # moved to partition 0
            bcast_sb = cp.tile([DIM_HEAD, 2, N], f32)
            out0_sb = cp.tile([128, 4, QUERY_DIM], f32)   # pair-0 proj + bias
            out_sb = cp.tile([128, 4, QUERY_DIM], f32)

            # prologue DMAs, ordered so the first production chunk and qT can
            # start as early as possible; the rest of the prologue (wv, ones,
            # wo, bias) is emitted after chunk 0's context DMA below.
            nc.sync.dma_start(out=wq_sb[:], in_=wq_r)
            nc.sync.dma_start(out=xt_sb[:], in_=xt_r)
            nc.sync.dma_start(out=wk_sb[:], in_=wk_r)

            # kT for pair 1 stays resident; pair 0's kT lives in rotating
            # chunk tiles consumed during pass 0.
            kt_f1 = ktp.tile([128, M], f32r, tag="ktf1", name="ktf1", bufs=1)
            kt_chunks = {}

            def produce_chunk(mc):
                """DMA chunk mc of contextT; return kT/v production emitters
                (closures) so production interleaves with attention tiles."""
                ct_t = ctp.tile([128, 2, MCHUNK], f32r, tag="ct", name=f"ct{mc}")
                ct_dma = nc.sync.dma_start(
                    out=ct_t[:], in_=ct_r[:, :, mc * MCHUNK:(mc + 1) * MCHUNK]
                )
                if mc >= 1:
                    # keep the small prologue DMAs ahead of the chunk stream
                    # on the SP queue
                    for d in late_dmas:
                        tile.add_dep_helper(ct_dma.ins, d.ins, sync=False,
                                            reason="prologue before ct stream")
                kt_t = ktp.tile([128, MCHUNK], f32r, tag="kt", name=f"kt{mc}")
                kt_chunks[mc] = kt_t

                def kt_group(pp):
                    def go():
                        kt_ps = scp.tile([128, 1024], f32, tag="sc",
                                         name=f"ktps{pp}{mc}")
                        for h2 in range(2):
                            for t in range(2):
                                nc.tensor.matmul(
                                    kt_ps[:, h2 * 512:(h2 + 1) * 512],
                                    lhsT=wk_sb[:, t, pp * 128:(pp + 1) * 128],
                                    rhs=ct_t[:, t, h2 * 512:(h2 + 1) * 512],
                                    start=(t == 0), stop=(t == 1),
                                    skip_group_check=True,
                                )
                        dst = (kt_t[:, :] if pp == 0 else
                               kt_f1[:, mc * MCHUNK:(mc + 1) * MCHUNK])
                        nc.vector.tensor_copy(dst, kt_ps[:])
                    return go

                def v_group(s4):
                    def go():
                        v_ps = scp.tile([128, 1024], f32, tag="sc",
                                        name=f"vps{mc}{s4}")
                        for q in range(4):
                            s = s4 * 4 + q
                            for t in range(2):
                                nc.tensor.matmul(
                                    v_ps[:, q * 256:(q + 1) * 256],
                                    lhsT=ct_t[:, t, s * 128:(s + 1) * 128],
                                    rhs=wv_sb[:, t, :],
                                    start=(t == 0), stop=(t == 1),
                                    skip_group_check=True,
                                )
                        nc.vector.tensor_copy(
                            v_full[:, mc * MT_PER_CHUNK + s4 * 4:
                                   mc * MT_PER_CHUNK + s4 * 4 + 4, :, 0:DIM_HEAD],
                            v_ps[:].rearrange("p (s h d) -> p s h d", s=4, h=HPC),
                        )
                    return go

                # order: pair-0 kT first (needed immediately), v next (needed
                # by AV shortly after), pair-1 kT last (pass 1 only)
                return [kt_group(0), v_group(0), v_group(1), kt_group(1)]

            def qk_exp(p, mi):
                sc = scp.tile([128, 1024], f32, tag="sc", name=f"sc{p}{mi}")
                ks = (kt_chunks[mi // MT_PER_CHUNK] if p == 0 else kt_f1)
                off = (mi % MT_PER_CHUNK if p == 0 else mi) * 128
                ks = ks[:, off:off + 128]
                # two heads in one PE pass via row tiling
                nc.tensor.matmul(sc[:, 0:512], lhsT=ks[0:64, :],
                                 rhs=qt_sb[0:64, p, :], start=True, stop=True)
                nc.tensor.matmul(sc[:, 512:1024], lhsT=ks[64:128, :],
                                 rhs=qt_sb[64:128, p, :], start=True, stop=True)
                e_t = ep.tile([128, 1024], f32r, tag="e", name=f"e{p}{mi}")
                nc.scalar.activation(e_t[:], sc[:], EXP, scale=SCALE)
                return e_t

            def av(p, mi, e_t, acc):
                for h2 in range(2):
                    nc.tensor.matmul(
                        acc[h2][0:DIM_HEAD + 1, :],
                        lhsT=v_full[:, mi, 2 * p + h2, :],
                        rhs=e_t[:, h2 * 512:(h2 + 1) * 512],
                        start=(mi == 0), stop=(mi == M // 128 - 1),
                        skip_group_check=True,
                    )

            def attention_tile(p, mi, acc):
                av(p, mi, qk_exp(p, mi), acc)

            def pass_tail(p, acc):
                """normalize numerators by the ones-row denominator; per-head
                chains so head 0's broadcast overlaps head 1's reciprocal"""
                for h2 in range(2):
                    nc.vector.reciprocal(
                        recip_sb[DIM_HEAD:DIM_HEAD + 1, h2, :],
                        acc[h2][DIM_HEAD:DIM_HEAD + 1, :],
                    )
                    # partition_broadcast on HW always reads partition 0 of
                    # the tensor, so shift the reciprocal row there first.
                    nc.sync.dma_start(
                        out=recip0_sb[0:1, h2, :],
                        in_=recip_sb[DIM_HEAD:DIM_HEAD + 1, h2, :],
                    )
                    nc.gpsimd.partition_broadcast(bcast_sb[:, h2, :],
                                                  recip0_sb[0:1, h2, :])
                    nc.vector.tensor_mul(
                        stack_sb[:, 2 * p + h2, :], acc[h2][0:DIM_HEAD, :],
                        bcast_sb[:, h2, :]
                    )

            # chunk-0 context DMA goes out right behind the qT weights
            chunk0 = produce_chunk(0)

            # late prologue (not needed until mid-kernel)
            late_dmas = []
            late_dmas.append(nc.sync.dma_start(out=wv_sb[:], in_=wv_r))
            late_dmas.append(nc.sync.dma_start(out=wo_sb[:], in_=wo[:, :, :]))
            late_dmas.append(nc.sync.dma_start(out=bo_sb[:], in_=bo2[:, :]))
            # ones column of v_aug: memset a [128, 1] column, then one
            # broadcast-copy into the strided ones slots (rounds to f32r)
            ones_col = cp.tile([128, 1], f32)
            nc.vector.memset(ones_col[:], 1.0)
            _oc, _vdst = bass.broadcast_tensor_aps(
                ones_col[:, :], v_full[:, :, :, DIM_HEAD].rearrange(
                    "p s h -> p (s h)")[:, None, :].rearrange("p o q -> p (o q)")
            )
            nc.vector.tensor_copy(_vdst, _oc)
            nc.gpsimd.partition_broadcast(bo_bc[:], bo_sb[0:1, :])

            # qT per head-pair p: [128, N]; rows 0-63 head 2p, 64-127 head 2p+1
            q_ps = scp.tile([128, 1024], f32, tag="sc", name="q_ps")
            for p in range(2):
                for t in range(4):
                    nc.tensor.matmul(
                        q_ps[:, p * 512:(p + 1) * 512],
                        lhsT=wq_sb[:, t, p * 128:(p + 1) * 128],
                        rhs=xt_sb[:, t, :],
                        start=(t == 0), stop=(t == 3),
                        skip_group_check=True,
                    )
            nc.vector.tensor_copy(
                qt_sb[:, :, :], q_ps[:].rearrange("p (a n) -> p a n", a=2))

            # ---- pass 0 (heads 0,1), production pipelined one chunk ahead --
            acc0 = [accp.tile([128, N], f32, tag=f"acc{h2}", name=f"a0{h2}")
                    for h2 in range(2)]
            prefetch = {}
            for step in range(NCHUNKS + 1):
                prod = (chunk0 if step == 0 else produce_chunk(step)) \
                    if step < NCHUNKS else []
                atts = (
                    list(range((step - 1) * MT_PER_CHUNK, step * MT_PER_CHUNK))
                    if step >= 1 else []
                )
                for i in range(max(2 * len(prod), len(atts))):
                    if i < len(atts):
                        attention_tile(0, atts[i], acc0)
                    if i % 2 == 0 and i // 2 < len(prod):
                        prod[i // 2]()
            pass_tail(0, acc0)

            # partial projection for pair 0 (+ bias) overlaps pass 1
            def proj_pair0():
                for g in range(2):
                    pr0 = scp.tile([128, 1024], f32, tag="sc", name=f"pr0{g}")
                    for j in range(2):
                        nt = g * 2 + j
                        for h in range(2):
                            nc.tensor.matmul(
                                pr0[:, j * 512:(j + 1) * 512],
                                lhsT=stack_sb[:, h, nt * 128:(nt + 1) * 128],
                                rhs=wo_sb[:, h, :],
                                start=(h == 0), stop=(h == 1),
                                skip_group_check=True,
                            )
                    for j in range(2):
                        nt = g * 2 + j
                        nc.vector.tensor_add(
                            out0_sb[:, nt, :], pr0[:, j * 512:(j + 1) * 512],
                            bo_bc[:])

            # ---- pass 1 (heads 2,3): pure attention from resident kT/v ----
            acc1 = [accp.tile([128, N], f32, tag=f"acc{h2}", name=f"a1{h2}")
                    for h2 in range(2)]
            for mi in range(M // 128):
                if mi in prefetch:
                    av(1, mi, prefetch.pop(mi), acc1)
                else:
                    attention_tile(1, mi, acc1)
                if mi == 8:
                    proj_pair0()
            pass_tail(1, acc1)

            # ---- pair-1 projection + combine + store ----
            for g in range(2):
                pr = scp.tile([128, 1024], f32, tag="sc", name=f"pr{g}")
                for j in range(2):
                    nt = g * 2 + j
                    for h in range(2, 4):
                        nc.tensor.matmul(
                            pr[:, j * 512:(j + 1) * 512],
                            lhsT=stack_sb[:, h, nt * 128:(nt + 1) * 128],
                            rhs=wo_sb[:, h, :],
                            start=(h == 2), stop=(h == 3),
                            skip_group_check=True,
                        )
                for j in range(2):
                    nt = g * 2 + j
                    nc.vector.tensor_add(
                        out_sb[:, nt, :], pr[:, j * 512:(j + 1) * 512],
                        out0_sb[:, nt, :])
                    nc.sync.dma_start(out=out_r[:, nt, :], in_=out_sb[:, nt, :])

    nc.compile()
    return nc


def _get_nc():
    if "nc" not in _CACHE:
        _CACHE["nc"] = _build_nc()
    return _CACHE["nc"]


def _make_in_maps(x, context, Wq, Wkv, Wo, bo):
    x = np.asarray(x, dtype=np.float32)
    context = np.asarray(context, dtype=np.float32)
    Wq = np.asarray(Wq, dtype=np.float32)
    Wkv = np.asarray(Wkv, dtype=np.float32)
    Wo = np.asarray(Wo, dtype=np.float32)
    bo = np.asarray(bo, dtype=np.float32)

    Wk = Wkv[:, :ATT_DIM]
    Wv = Wkv[:, ATT_DIM:]
    bo2 = np.ascontiguousarray((bo / 2.0)[None, :])

    in_maps = []
    for c in range(N_CORES):
        b, g = divmod(c, 2)
        hs = g * HPC * DIM_HEAD           # column offset of this core's heads
        he = hs + HPC * DIM_HEAD
        wo_core = Wo[hs:he, :].reshape(HPC, DIM_HEAD, QUERY_DIM)
        in_maps.append({
            "ct": np.ascontiguousarray(context[b].T),
            "xt": np.ascontiguousarray(x[b].T),
            "wq": np.ascontiguousarray(Wq[:, hs:he]),
            "wk": np.ascontiguousarray(Wk[:, hs:he]),
            "wv": np.ascontiguousarray(Wv[:, hs:he]),
            "wo": np.ascontiguousarray(wo_core.transpose(1, 0, 2)),
            "bo2": bo2,
        })
    return in_maps


def run(inputs, trace=False, **spmd_kwargs):
    """Run the kernel; returns (full_output [B,N,QUERY_DIM], BassKernelResults)."""
    from concourse.bass_utils import run_bass_kernel_spmd

    nc = _get_nc()
    in_maps = _make_in_maps(**inputs)
    res = run_bass_kernel_spmd(
        nc, in_maps, core_ids=list(range(N_CORES)), trace=trace, **spmd_kwargs
    )
    outs = [r["out"] for r in res.results]
    full = np.empty((B, N, QUERY_DIM), dtype=np.float32)
    for b in range(B):
        full[b] = outs[2 * b] + outs[2 * b + 1]
    return full, res


def kernel(**inputs) -> np.ndarray:
    full, _ = run(inputs, trace=False)
    return full


# revision 39
# speedup vs baseline: 1.0167x; 1.0159x over previous
# Bass/Tile Trainium2 kernel for nn_Attention_48816598286380.
#
# Reference computation (B=4, N=512, M=8192, Hq=512, Ck=256, H=8, D=64):
#   q = x @ Wq;  k,v = split(context @ Wkv);  per-head softmax(q k^T / sqrt(D)) v
#   out = attn_out @ Wo + bo
#
# Sharding: 8 cores = 4 batches x 2 head-groups (4 heads each).  Each core
# computes its batch's attention for its 4 heads plus the partial output
# projection over those heads; the host sums the two partial projections per
# batch (pure unshard of a sum-sharded tensor; bo is split half/half so the
# sum carries the full bias).
#
# On-device layout: everything is computed in "transposed" orientation so all
# matmul contractions sit on the partition axis:
#   qT[d, n], kT[d, m] from xT / contextT (host supplies the transposes)
#   scoresT[m, n] = kT(m-tile).T @ qT      (two heads packed via PE row tiling)
#   E = exp(scoresT / 8)  on ScalarE, PSUM -> SBUF, [128, 1024] per instr
#   numerT[d, n] (+ denominator row 64) = v_aug(m-tile).T @ E  accumulated in
#     PSUM, where v_aug = [v | ones], so the softmax denominator falls out of
#     the same matmul.
#   out_partial[n, f] = sum_h (numerT_h / den_h) contracted with Wo rows.
#
# All matmul-feeding tensors are declared float32r (full-rate fp32 path on
# the PE; plain fp32 runs at 1/4 rate; walrus requires producers to declare
# f32r output).  Two passes over m (one per head pair) keep the PSUM
# budget at 8 banks; kT/v production is software-pipelined one chunk ahead of
# the attention loop, and pair 1's kT plus all of v stay resident in SBUF so
# pass 1 needs no DMA or production work.

import numpy as np

B, N, M = 4, 512, 8192
QUERY_DIM, INPUT_DIM = 512, 256
HEADS, DIM_HEAD = 8, 64
ATT_DIM = HEADS * DIM_HEAD  # 512
HPC = 4          # heads per core
N_CORES = 8
MCHUNK = 1024    # context rows produced per pipeline step
NCHUNKS = M // MCHUNK
MT_PER_CHUNK = MCHUNK // 128
SCALE = DIM_HEAD ** -0.5

_CACHE = {}


def _build_nc():
    import concourse.bacc as bacc
    import concourse.bass as bass
    import concourse.mybir as mybir
    import concourse.tile as tile

    f32 = mybir.dt.float32
    f32r = mybir.dt.float32r
    EXP = mybir.ActivationFunctionType.Exp

    nc = bacc.Bacc(None, target_bir_lowering=False)

    ct = nc.dram_tensor("ct", [INPUT_DIM, M], f32r, kind="ExternalInput")  # context[b].T
    xt = nc.dram_tensor("xt", [QUERY_DIM, N], f32r, kind="ExternalInput")  # x[b].T
    wq = nc.dram_tensor("wq", [QUERY_DIM, HPC * DIM_HEAD], f32r, kind="ExternalInput")
    wk = nc.dram_tensor("wk", [INPUT_DIM, HPC * DIM_HEAD], f32r, kind="ExternalInput")
    wv = nc.dram_tensor("wv", [INPUT_DIM, HPC * DIM_HEAD], f32r, kind="ExternalInput")
    wo = nc.dram_tensor("wo", [DIM_HEAD, HPC, QUERY_DIM], f32r, kind="ExternalInput")
    bo2 = nc.dram_tensor("bo2", [1, QUERY_DIM], f32, kind="ExternalInput")  # bo / 2
    out = nc.dram_tensor("out", [N, QUERY_DIM], f32, kind="ExternalOutput")

    ct_r = ct[:, :].rearrange("(t p) m -> p t m", p=128)    # [128, 2, M]
    xt_r = xt[:, :].rearrange("(t p) n -> p t n", p=128)    # [128, 4, N]
    wq_r = wq[:, :].rearrange("(t p) d -> p t d", p=128)    # [128, 4, 256]
    wk_r = wk[:, :].rearrange("(t p) d -> p t d", p=128)    # [128, 2, 256]
    wv_r = wv[:, :].rearrange("(t p) d -> p t d", p=128)    # [128, 2, 256]
    out_r = out[:, :].rearrange("(t p) f -> p t f", p=128)  # [128, 4, 512]

    with tile.TileContext(nc) as tc:
        with (
            tc.tile_pool(name="const", bufs=1) as cp,
            tc.tile_pool(name="ctp", bufs=2) as ctp,
            tc.tile_pool(name="ktp", bufs=2) as ktp,
            tc.tile_pool(name="ep", bufs=4) as ep,
            tc.tile_pool(name="scp", bufs=3, space="PSUM") as scp,
            tc.tile_pool(name="accp", bufs=1, space="PSUM") as accp,
        ):
            # ---- constants ----
            xt_sb = cp.tile([128, 4, N], f32r)
            wq_sb = cp.tile([128, 4, HPC * DIM_HEAD], f32r)
            wk_sb = cp.tile([128, 2, HPC * DIM_HEAD], f32r)
            wv_sb = cp.tile([128, 2, HPC * DIM_HEAD], f32r)
            wo_sb = cp.tile([DIM_HEAD, HPC, QUERY_DIM], f32r)
            bo_sb = cp.tile([1, QUERY_DIM], f32)
            bo_bc = cp.tile([128, QUERY_DIM], f32)
            qt_sb = cp.tile([128, 2, N], f32r)
            # v for all 4 heads, all of M, with a ones column per head:
            # [128 (m within tile), m-tile, head, 64 v | 1 one]
            v_full = cp.tile([128, M // 128, HPC, DIM_HEAD + 1], f32r)
            stack_sb = cp.tile([DIM_HEAD, HPC, N], f32r)  # normalized attn outT
            recip_sb = cp.tile([128, 2, N], f32)          # partition 64, per pass
            recip0_sb = cp.tile([1, 2, N], f32)           # moved to partition 0
            bcast_sb = cp.tile([DIM_HEAD, 2, N], f32)
            out0_sb = cp.tile([128, 4, QUERY_DIM], f32)   # pair-0 proj + bias
            out_sb = cp.tile([128, 4, QUERY_DIM], f32)

            # prologue DMAs, ordered so the first production chunk and qT can
            # start as early as possible; the rest of the prologue (wv, ones,
            # wo, bias) is emitted after chunk 0's context DMA below.
            nc.sync.dma_start(out=wq_sb[:], in_=wq_r)
            nc.sync.dma_start(out=xt_sb[:], in_=xt_r)
            nc.sync.dma_start(out=wk_sb[:], in_=wk_r)

            # PE warm-up: the HAM clock gate holds the PE at 1.2 GHz until
            # ~3.4 us of sustained activity.  Run throwaway matmuls on a
            # zeroed tile while the prologue DMAs are in flight so qT/kT and
            # the first score tiles run at full clock.
            warm_sb = cp.tile([128, 64], f32)
            nc.vector.memset(warm_sb[:], 0.0)
            warm_ps = scp.tile([128, 1024], f32, tag="sc", name="warm_ps")
            for w in range(24):
                nc.tensor.matmul(
                    warm_ps[0:64, 0:64], lhsT=warm_sb[:], rhs=warm_sb[:],
                    start=True, stop=True, skip_group_check=True,
                )

            # kT for pair 1 stays resident; pair 0's kT lives in rotating
            # chunk tiles consumed during pass 0.
            kt_f1 = ktp.tile([128, M], f32r, tag="ktf1", name="ktf1", bufs=1)
            kt_chunks = {}

            def produce_chunk(mc):
                """DMA chunk mc of contextT; return kT/v production emitters
                (closures) so production interleaves with attention tiles."""
                ct_t = ctp.tile([128, 2, MCHUNK], f32r, tag="ct", name=f"ct{mc}")
                ct_dma = nc.sync.dma_start(
                    out=ct_t[:], in_=ct_r[:, :, mc * MCHUNK:(mc + 1) * MCHUNK]
                )
                if mc >= 1:
                    # keep the small prologue DMAs ahead of the chunk stream
                    # on the SP queue
                    for d in late_dmas:
                        tile.add_dep_helper(ct_dma.ins, d.ins, sync=False,
                                            reason="prologue before ct stream")
                kt_t = ktp.tile([128, MCHUNK], f32r, tag="kt", name=f"kt{mc}")
                kt_chunks[mc] = kt_t

                def kt_group(pp):
                    def go():
                        kt_ps = scp.tile([128, 1024], f32, tag="sc",
                                         name=f"ktps{pp}{mc}")
                        for h2 in range(2):
                            for t in range(2):
                                nc.tensor.matmul(
                                    kt_ps[:, h2 * 512:(h2 + 1) * 512],
                                    lhsT=wk_sb[:, t, pp * 128:(pp + 1) * 128],
                                    rhs=ct_t[:, t, h2 * 512:(h2 + 1) * 512],
                                    start=(t == 0), stop=(t == 1),
                                    skip_group_check=True,
                                )
                        dst = (kt_t[:, :] if pp == 0 else
                               kt_f1[:, mc * MCHUNK:(mc + 1) * MCHUNK])
                        nc.vector.tensor_copy(dst, kt_ps[:])
                    return go

                def v_group(s4):
                    def go():
                        v_ps = scp.tile([128, 1024], f32, tag="sc",
                                        name=f"vps{mc}{s4}")
                        for q in range(4):
                            s = s4 * 4 + q
                            for t in range(2):
                                nc.tensor.matmul(
                                    v_ps[:, q * 256:(q + 1) * 256],
                                    lhsT=ct_t[:, t, s * 128:(s + 1) * 128],
                                    rhs=wv_sb[:, t, :],
                                    start=(t == 0), stop=(t == 1),
                                    skip_group_check=True,
                                )
                        nc.vector.tensor_copy(
                            v_full[:, mc * MT_PER_CHUNK + s4 * 4:
                                   mc * MT_PER_CHUNK + s4 * 4 + 4, :, 0:DIM_HEAD],
                            v_ps[:].rearrange("p (s h d) -> p s h d", s=4, h=HPC),
                        )
                    return go

                # order: pair-0 kT first (needed immediately), v next (needed
                # by AV shortly after), pair-1 kT last (pass 1 only)
                return [kt_group(0), v_group(0), v_group(1), kt_group(1)]

            def qk_exp(p, mi):
                sc = scp.tile([128, 1024], f32, tag="sc", name=f"sc{p}{mi}")
                ks = (kt_chunks[mi // MT_PER_CHUNK] if p == 0 else kt_f1)
                off = (mi % MT_PER_CHUNK if p == 0 else mi) * 128
                ks = ks[:, off:off + 128]
                # two heads in one PE pass via row tiling
                nc.tensor.matmul(sc[:, 0:512], lhsT=ks[0:64, :],
                                 rhs=qt_sb[0:64, p, :], start=True, stop=True)
                nc.tensor.matmul(sc[:, 512:1024], lhsT=ks[64:128, :],
                                 rhs=qt_sb[64:128, p, :], start=True, stop=True)
                e_t = ep.tile([128, 1024], f32r, tag="e", name=f"e{p}{mi}")
                nc.scalar.activation(e_t[:], sc[:], EXP, scale=SCALE)
                return e_t

            def av(p, mi, e_t, acc):
                for h2 in range(2):
                    nc.tensor.matmul(
                        acc[h2][0:DIM_HEAD + 1, :],
                        lhsT=v_full[:, mi, 2 * p + h2, :],
                        rhs=e_t[:, h2 * 512:(h2 + 1) * 512],
                        start=(mi == 0), stop=(mi == M // 128 - 1),
                        skip_group_check=True,
                    )

            def attention_tile(p, mi, acc):
                av(p, mi, qk_exp(p, mi), acc)

            def pass_tail(p, acc):
                """normalize numerators by the ones-row denominator; per-head
                chains so head 0's broadcast overlaps head 1's reciprocal"""
                for h2 in range(2):
                    nc.vector.reciprocal(
                        recip_sb[DIM_HEAD:DIM_HEAD + 1, h2, :],
                        acc[h2][DIM_HEAD:DIM_HEAD + 1, :],
                    )
                    # partition_broadcast on HW always reads partition 0 of
                    # the tensor, so shift the reciprocal row there first.
                    nc.sync.dma_start(
                        out=recip0_sb[0:1, h2, :],
                        in_=recip_sb[DIM_HEAD:DIM_HEAD + 1, h2, :],
                    )
                    nc.gpsimd.partition_broadcast(bcast_sb[:, h2, :],
                                                  recip0_sb[0:1, h2, :])
                    nc.vector.tensor_mul(
                        stack_sb[:, 2 * p + h2, :], acc[h2][0:DIM_HEAD, :],
                        bcast_sb[:, h2, :]
                    )

            # chunk-0 context DMA goes out right behind the qT weights
            chunk0 = produce_chunk(0)

            # late prologue (not needed until mid-kernel)
            late_dmas = []
            late_dmas.append(nc.sync.dma_start(out=wv_sb[:], in_=wv_r))
            late_dmas.append(nc.sync.dma_start(out=wo_sb[:], in_=wo[:, :, :]))
            late_dmas.append(nc.sync.dma_start(out=bo_sb[:], in_=bo2[:, :]))
            # ones column of v_aug: memset a [128, 1] column, then one
            # broadcast-copy into the strided ones slots (rounds to f32r)
            ones_col = cp.tile([128, 1], f32)
            nc.vector.memset(ones_col[:], 1.0)
            _oc, _vdst = bass.broadcast_tensor_aps(
                ones_col[:, :], v_full[:, :, :, DIM_HEAD].rearrange(
                    "p s h -> p (s h)")[:, None, :].rearrange("p o q -> p (o q)")
            )
            nc.vector.tensor_copy(_vdst, _oc)
            nc.gpsimd.partition_broadcast(bo_bc[:], bo_sb[0:1, :])

            # qT per head-pair p: [128, N]; rows 0-63 head 2p, 64-127 head 2p+1
            q_ps = scp.tile([128, 1024], f32, tag="sc", name="q_ps")
            for p in range(2):
                for t in range(4):
                    nc.tensor.matmul(
                        q_ps[:, p * 512:(p + 1) * 512],
                        lhsT=wq_sb[:, t, p * 128:(p + 1) * 128],
                        rhs=xt_sb[:, t, :],
                        start=(t == 0), stop=(t == 3),
                        skip_group_check=True,
                    )
            nc.vector.tensor_copy(
                qt_sb[:, :, :], q_ps[:].rearrange("p (a n) -> p a n", a=2))

            # ---- pass 0 (heads 0,1), production pipelined one chunk ahead --
            acc0 = [accp.tile([128, N], f32, tag=f"acc{h2}", name=f"a0{h2}")
                    for h2 in range(2)]
            prefetch = {}
            for step in range(NCHUNKS + 1):
                prod = (chunk0 if step == 0 else produce_chunk(step)) \
                    if step < NCHUNKS else []
                atts = (
                    list(range((step - 1) * MT_PER_CHUNK, step * MT_PER_CHUNK))
                    if step >= 1 else []
                )
                for i in range(max(2 * len(prod), len(atts))):
                    if i < len(atts):
                        attention_tile(0, atts[i], acc0)
                    if i % 2 == 0 and i // 2 < len(prod):
                        prod[i // 2]()
            pass_tail(0, acc0)

            # partial projection for pair 0 (+ bias) overlaps pass 1
            def proj_pair0():
                for g in range(2):
                    pr0 = scp.tile([128, 1024], f32, tag="sc", name=f"pr0{g}")
                    for j in range(2):
                        nt = g * 2 + j
                        for h in range(2):
                            nc.tensor.matmul(
                                pr0[:, j * 512:(j + 1) * 512],
                                lhsT=stack_sb[:, h, nt * 128:(nt + 1) * 128],
                                rhs=wo_sb[:, h, :],
                                start=(h == 0), stop=(h == 1),
                                skip_group_check=True,
                            )
                    for j in range(2):
                        nt = g * 2 + j
                        nc.vector.tensor_add(
                            out0_sb[:, nt, :], pr0[:, j * 512:(j + 1) * 512],
                            bo_bc[:])

            # ---- pass 1 (heads 2,3): pure attention from resident kT/v ----
            acc1 = [accp.tile([128, N], f32, tag=f"acc{h2}", name=f"a1{h2}")
                    for h2 in range(2)]
            for mi in range(M // 128):
                if mi in prefetch:
                    av(1, mi, prefetch.pop(mi), acc1)
                else:
                    attention_tile(1, mi, acc1)
                if mi == 8:
                    proj_pair0()
            pass_tail(1, acc1)

            # ---- pair-1 projection + combine + store ----
            for g in range(2):
                pr = scp.tile([128, 1024], f32, tag="sc", name=f"pr{g}")
                for j in range(2):
                    nt = g * 2 + j
                    for h in range(2, 4):
                        nc.tensor.matmul(
                            pr[:, j * 512:(j + 1) * 512],
                            lhsT=stack_sb[:, h, nt * 128:(nt + 1) * 128],
                            rhs=wo_sb[:, h, :],
                            start=(h == 2), stop=(h == 3),
                            skip_group_check=True,
                        )
                for j in range(2):
                    nt = g * 2 + j
                    nc.vector.tensor_add(
                        out_sb[:, nt, :], pr[:, j * 512:(j + 1) * 512],
                        out0_sb[:, nt, :])
                    nc.sync.dma_start(out=out_r[:, nt, :], in_=out_sb[:, nt, :])

    nc.compile()
    return nc


def _get_nc():
    if "nc" not in _CACHE:
        _CACHE["nc"] = _build_nc()
    return _CACHE["nc"]


def _make_in_maps(x, context, Wq, Wkv, Wo, bo):
    x = np.asarray(x, dtype=np.float32)
    context = np.asarray(context, dtype=np.float32)
    Wq = np.asarray(Wq, dtype=np.float32)
    Wkv = np.asarray(Wkv, dtype=np.float32)
    Wo = np.asarray(Wo, dtype=np.float32)
    bo = np.asarray(bo, dtype=np.float32)

    Wk = Wkv[:, :ATT_DIM]
    Wv = Wkv[:, ATT_DIM:]
    bo2 = np.ascontiguousarray((bo / 2.0)[None, :])

    in_maps = []
    for c in range(N_CORES):
        b, g = divmod(c, 2)
        hs = g * HPC * DIM_HEAD           # column offset of this core's heads
        he = hs + HPC * DIM_HEAD
        wo_core = Wo[hs:he, :].reshape(HPC, DIM_HEAD, QUERY_DIM)
        in_maps.append({
            "ct": np.ascontiguousarray(context[b].T),
            "xt": np.ascontiguousarray(x[b].T),
            "wq": np.ascontiguousarray(Wq[:, hs:he]),
            "wk": np.ascontiguousarray(Wk[:, hs:he]),
            "wv": np.ascontiguousarray(Wv[:, hs:he]),
            "wo": np.ascontiguousarray(wo_core.transpose(1, 0, 2)),
            "bo2": bo2,
        })
    return in_maps


def run(inputs, trace=False, **spmd_kwargs):
    """Run the kernel; returns (full_output [B,N,QUERY_DIM], BassKernelResults)."""
    from concourse.bass_utils import run_bass_kernel_spmd

    nc = _get_nc()
    in_maps = _make_in_maps(**inputs)
    res = run_bass_kernel_spmd(
        nc, in_maps, core_ids=list(range(N_CORES)), trace=trace, **spmd_kwargs
    )
    outs = [r["out"] for r in res.results]
    full = np.empty((B, N, QUERY_DIM), dtype=np.float32)
    for b in range(B):
        full[b] = outs[2 * b] + outs[2 * b + 1]
    return full, res


def kernel(**inputs) -> np.ndarray:
    full, _ = run(inputs, trace=False)
    return full
